# revision 12
# baseline (speedup 1.0000x reference)
"""GCNConv (add self-loops, symmetric norm, linear, relu, broadcast) on 8 TRN2 cores.

Hybrid dense/sparse aggregation, no cross-core communication:

- Destination nodes are row-sharded across the 8 cores (1250 rows each).
- Phase 0 (replicated): every core computes h = x @ W (x supplied
  pre-transposed/padded).  The first NBS source blocks ("sparse range")
  are written to local DRAM; the last D blocks ("dense range") stay
  resident in SBUF.
- Phase 1, per 128-row destination tile:
    sparse range: host-deduplicated source rows are gathered with
      prepare_only dma_gather (Q7 descriptor generation runs from t=0,
      overlapping phase 0; triggers fire once the sparse h rows land)
      and reduced with PE matmuls against host-built scatter blocks
      S[u, d] = sum of edge norms from gathered slot u into dst d.
    dense range: a host-built dense block adjacency A[s, d] is streamed
      from DRAM (HWDGE, no per-row descriptors) and reduced with PE
      matmuls against the SBUF-resident h blocks.
  Both accumulate into the same PSUM tile; bias-add + relu on DVE.

The dense/sparse split ratio balances the PE (matmul) against the Q7
(SWDGE descriptor emission) and HBM, which otherwise bottleneck the
all-sparse / all-dense variants respectively.
"""

import numpy as np
import ml_dtypes

import concourse.bacc as bacc
import concourse.mybir as mybir
import concourse.tile as tile
from concourse.bass_utils import run_bass_kernel_spmd

N_NODES = 10000
N_GENES = 978
EMBED = 301
HEADS = 4
REP = 12
N_CORES = 8
NPC = N_NODES // N_CORES          # 1250 dst rows per core
DT = 128                          # dst tile height
NT = (NPC + DT - 1) // DT         # 10 dst tiles per core
GP = 1024                         # padded gene dim (8 chunks of 128)
GCH = GP // 128
NB = 79                           # src blocks of 128 (79*128 = 10112 >= 10000)
SP = NB * 128
D_DENSE = 32                      # dense src blocks (tail of the range)
NBS = NB - D_DENSE                # sparse src blocks
SPR = NBS * 128                   # sparse rows
HROW = 384                        # gather elem: 384 f16 = 768 B (mult of 256)
G_BUFS = 9                        # gather slot pool depth
PREP_BATCH = 4                    # preps per trigger round (ring capacity)
SG = 512                          # xT stream chunk (free dim)
PREP_MODE = False                  # prepare_only + trigger vs plain gathers

F32 = mybir.dt.float32
F16 = mybir.dt.float16
I16 = mybir.dt.int16

_prog_cache: dict = {}


def _build_program(bmax: int):
    slots = bmax * 128
    nc = bacc.Bacc("TRN2", target_bir_lowering=False, debug=False,
                   num_devices=N_CORES, num_swdge_queues=4)

    xT_d = nc.dram_tensor("xT", [GP, SP], F16, kind="ExternalInput")
    W_d = nc.dram_tensor("Wp", [GP, EMBED], F16, kind="ExternalInput")
    b_d = nc.dram_tensor("bB", [128, EMBED], F32, kind="ExternalInput")
    S_d = nc.dram_tensor("Sblk", [NT, 128, slots], F16, kind="ExternalInput")
    ix_d = nc.dram_tensor("idxw", [NT, 128, slots // 16], I16, kind="ExternalInput")
    A_d = nc.dram_tensor("Adns", [NT, 128, D_DENSE, 128], F16, kind="ExternalInput")
    out_d = nc.dram_tensor("out", [NPC, EMBED], F16, kind="ExternalOutput")
    hsp_d = nc.dram_tensor("hsp", [SPR, HROW], F16)   # sparse-range h rows

    gsems = [nc.alloc_semaphore(f"gsem{t}") for t in range(NT)]

    with tile.TileContext(nc) as tc:
        if PREP_MODE:
            # clear gather sems on the engine that waits on them (PE): the
            # clears retire before phase 0's matmuls, long before any bump
            for s in gsems:
                nc.tensor.sem_clear(s)
        with (
            tc.tile_pool(name="const", bufs=1) as cpool,
            tc.tile_pool(name="sI", bufs=NT) as ipool,
            tc.tile_pool(name="sG", bufs=G_BUFS) as gpool,
            tc.tile_pool(name="sS", bufs=2) as spool,
            tc.tile_pool(name="sA", bufs=2) as apool,
            tc.tile_pool(name="sO", bufs=3) as opool,
            tc.tile_pool(name="pO", bufs=4, space="PSUM") as popool,
        ):
            b_sb = cpool.tile([128, EMBED], F32, tag="bias")
            nc.sync.dma_start(b_sb[:], b_d[:])
            hres = cpool.tile([128, D_DENSE, EMBED], F16, tag="hres")

            # idx tiles first: preps read them at desc-gen time
            ix_sbs = []
            for t in range(NT):
                ix_sb = ipool.tile([128, slots // 16], I16, tag="ix")
                nc.sync.dma_start(ix_sb[:], ix_d[t])
                ix_sbs.append(ix_sb)

            # prepare_only gathers: emitted after phase 0 (so Tile sees
            # write->read on hsp_d) but the Q7 starts descriptor emission
            # at ~t=0 since preps only depend on the idx tiles; the DMA
            # fires at trigger time (after sparse h lands).
            g_sbs = {}

            def prep_tile(t):
                g_sb = gpool.tile([128, bmax, HROW], F16, tag="g")
                nc.gpsimd.dma_gather(
                    g_sb[:], hsp_d[:], ix_sbs[t][:],
                    num_idxs=slots, num_idxs_reg=slots,
                    elem_size=HROW, queue_num=t % 4,
                    prepare_only=True, sem=gsems[t],
                )
                g_sbs[t] = g_sb

            # ---------------- phase 0: h = x @ W ----------------
            with (
                tc.tile_pool(name="wsb", bufs=1) as wpool,
                tc.tile_pool(name="xt", bufs=3) as xpool,
                tc.tile_pool(name="hsb", bufs=4) as hpool,
                tc.tile_pool(name="ph", bufs=4, space="PSUM") as phpool,
            ):
                w_sb = wpool.tile([128, GCH, EMBED], F16)
                for g in range(GCH):
                    nc.sync.dma_start(w_sb[:, g, :], W_d[g * 128:(g + 1) * 128, :])

                h_writes = []
                for s0 in range(0, SP, SG):
                    sgw = min(SG, SP - s0)
                    xt = xpool.tile([128, GCH, SG], F16, tag="xt")
                    nc.sync.dma_start(xt[:, :, :sgw],
                                      xT_d[:].rearrange("(g p) n -> p g n", p=128)
                                      [:, :, s0:s0 + sgw])
                    for sub in range(sgw // 128):
                        blk = (s0 + sub * 128) // 128
                        ph = phpool.tile([128, EMBED], F32)
                        for g in range(GCH):
                            nc.tensor.matmul(
                                ph[:],
                                xt[:, g, sub * 128:(sub + 1) * 128],
                                w_sb[:, g, :],
                                start=(g == 0), stop=(g == GCH - 1),
                            )
                        if blk < NBS:
                            h_sb = hpool.tile([128, EMBED], F16, tag="h")
                            nc.vector.tensor_copy(h_sb[:], ph[:])
                            r = blk * 128
                            h_writes.append(nc.scalar.dma_start(
                                hsp_d[r:r + 128, :EMBED], h_sb[:]))
                        else:
                            nc.vector.tensor_copy(hres[:, blk - NBS, :], ph[:])

                sent = nc.sync.nop()
                for hw in h_writes:
                    tile.add_dep_helper(sent.ins, hw.ins, reason="h-sparse ready")

            # preps + triggers in rounds bounded by the SWDGE ring capacity
            # (untriggered descriptors cannot be reclaimed)
            if PREP_MODE:
                for t0 in range(0, NT, PREP_BATCH):
                    batch = list(range(t0, min(t0 + PREP_BATCH, NT)))
                    for t in batch:
                        prep_tile(t)
                    for q in sorted({t % 4 for t in batch}):
                        trg = nc.gpsimd.trigger_dma(count=None, queue_num=q)
                        tile.add_dep_helper(trg.ins, sent.ins,
                                            reason="trigger waits h")
            else:
                GBLK = 8
                qctr = 0
                for t in range(NT):
                    g_sb = gpool.tile([128, bmax, HROW], F16, tag="g")
                    for b0 in range(0, bmax, GBLK):
                        nb = min(GBLK, bmax - b0)
                        gi = nc.gpsimd.dma_gather(
                            g_sb[:, b0:b0 + nb, :], hsp_d[:],
                            ix_sbs[t][:, b0 * 8:(b0 + nb) * 8],
                            num_idxs=nb * 128, num_idxs_reg=nb * 128,
                            elem_size=HROW, queue_num=qctr % 4,
                        )
                        qctr += 1
                        tile.add_dep_helper(gi.ins, sent.ins,
                                            reason="gather waits h")
                    g_sbs[t] = g_sb

            # ------------- phase 1: S-matmul + dense + bias/relu -------
            pre_s = {}
            pre_a = {}
            for t in range(2):
                s_sb = spool.tile([128, slots], F16, tag="s")
                nc.sync.dma_start(s_sb[:], S_d[t])
                pre_s[t] = s_sb
                a_sb = apool.tile([128, D_DENSE, 128], F16, tag="a")
                nc.sync.dma_start(a_sb[:], A_d[t])
                pre_a[t] = a_sb

            for t in range(NT):
                r0 = t * DT
                nr = min(DT, NPC - r0)
                s_sb = pre_s.pop(t)
                a_sb = pre_a.pop(t)
                tn = t + 2
                if tn < NT:
                    s_nx = spool.tile([128, slots], F16, tag="s")
                    nc.sync.dma_start(s_nx[:], S_d[tn])
                    pre_s[tn] = s_nx
                    a_nx = apool.tile([128, D_DENSE, 128], F16, tag="a")
                    nc.sync.dma_start(a_nx[:], A_d[tn])
                    pre_a[tn] = a_nx

                po = popool.tile([128, EMBED], F32)
                g_sb = g_sbs.pop(t)
                if PREP_MODE:
                    # gather data guard: prep-mode DMA completion is
                    # signalled only via the per-tile sem
                    nc.tensor.wait_ge(gsems[t], 16)
                for blk in range(bmax):
                    nc.tensor.matmul(
                        po[:],
                        s_sb[:, blk * 128:(blk + 1) * 128],
                        g_sb[:, blk, :EMBED],
                        start=(blk == 0), stop=False,
                    )
                for j in range(D_DENSE):
                    nc.tensor.matmul(
                        po[:],
                        a_sb[:, j, :],
                        hres[:, j, :],
                        start=False, stop=(j == D_DENSE - 1),
                    )
                o_sm = opool.tile([128, EMBED], F32, tag="osm")
                nc.vector.tensor_add(o_sm[:], po[:], b_sb[:])
                nc.vector.tensor_relu(o_sm[:], o_sm[:])
                o_cast = opool.tile([128, EMBED], F16, tag="ocast")
                nc.vector.tensor_copy(o_cast[:], o_sm[:])
                nc.scalar.dma_start(out_d[r0:r0 + nr, :], o_cast[:nr, :])

    nc.compile()
    return nc


def _preprocess(x, edge_index, edge_weight, W, b):
    src = np.concatenate([edge_index[0].astype(np.int64),
                          np.arange(N_NODES, dtype=np.int64)])
    dst = np.concatenate([edge_index[1].astype(np.int64),
                          np.arange(N_NODES, dtype=np.int64)])
    wf = np.concatenate([edge_weight.astype(np.float32),
                         np.ones(N_NODES, np.float32)])

    deg = np.bincount(dst, weights=wf.astype(np.float64),
                      minlength=N_NODES).astype(np.float32)
    dis = np.where(deg > 0, 1.0 / np.sqrt(deg), 0.0).astype(np.float32)
    norm = (dis[src] * wf * dis[dst]).astype(np.float32)

    order = np.argsort(dst, kind="stable")
    src_s, dst_s, norm_s = src[order], dst[order], norm[order]

    core_of = dst_s // NPC
    tloc_of = (dst_s % NPC) // DT
    group = core_of * NT + tloc_of
    cnt = np.bincount(group, minlength=N_CORES * NT)
    gstart = np.zeros(N_CORES * NT + 1, np.int64)
    gstart[1:] = np.cumsum(cnt)
    dloc = (dst_s % NPC) % DT

    sparse_mask = src_s < SPR

    # sparse range: dedup per (core, tile)
    uniq = []
    max_u = 0
    for g in range(N_CORES * NT):
        lo, hi = gstart[g], gstart[g + 1]
        m = sparse_mask[lo:hi]
        u, inv = np.unique(src_s[lo:hi][m], return_inverse=True)
        uniq.append((g // NT, g % NT, u, inv, lo, hi, m))
        max_u = max(max_u, len(u))
    bmax = max(1, (max_u + 127) // 128)
    slots = bmax * 128

    idx_arr = np.zeros((N_CORES, NT, slots), np.int16)
    S_f32 = np.zeros((N_CORES, NT, 128, slots), np.float32)
    A_f32 = np.zeros((N_CORES, NT, 128, D_DENSE, 128), np.float32)
    for k, t, u, inv, lo, hi, m in uniq:
        idx_arr[k, t, :len(u)] = u.astype(np.int16)
        np.add.at(S_f32[k, t],
                  (inv % 128, (inv // 128) * 128 + dloc[lo:hi][m]),
                  norm_s[lo:hi][m])
        # dense range
        sd = src_s[lo:hi][~m] - SPR
        np.add.at(A_f32[k, t], (sd % 128, sd // 128, dloc[lo:hi][~m]),
                  norm_s[lo:hi][~m])
    S_arr = S_f32.astype(np.float16)
    A_arr = A_f32.astype(np.float16)

    # SWDGE index layout: idx i lives at (partition i%16, col i//16),
    # replicated across the 8 sixteen-partition groups.
    cols = np.arange(slots // 16)
    idx_w = np.empty((N_CORES, NT, 128, slots // 16), np.int16)
    for p in range(16):
        lane = idx_arr[:, :, cols * 16 + p]
        idx_w[:, :, p::16, :] = lane[:, :, None, :]

    xT = np.zeros((GP, SP), np.float16)
    xT[:N_GENES, :N_NODES] = np.ascontiguousarray(
        x.astype(np.float32).T).astype(np.float16)
    Wp = np.zeros((GP, EMBED), np.float16)
    Wp[:N_GENES] = W.astype(np.float32).astype(np.float16)
    bB = np.broadcast_to(b.astype(np.float32), (128, EMBED)).copy()
    return xT, Wp, bB, S_arr, idx_w, A_arr, bmax


def make_in_maps(x, edge_index, edge_weight, W, b):
    xT, Wp, bB, S_arr, idx_w, A_arr, bmax = _preprocess(
        x, edge_index, edge_weight, W, b)
    in_maps = [
        {"xT": xT, "Wp": Wp, "bB": bB, "Sblk": S_arr[k], "idxw": idx_w[k],
         "Adns": A_arr[k]}
        for k in range(N_CORES)
    ]
    return in_maps, bmax


def get_program(bmax):
    if bmax not in _prog_cache:
        _prog_cache[bmax] = _build_program(bmax)
    return _prog_cache[bmax]


def kernel(x, edge_index, edge_weight, W, b):
    x = np.asarray(x)
    edge_index = np.asarray(edge_index)
    edge_weight = np.asarray(edge_weight)
    W = np.asarray(W)
    b = np.asarray(b)

    in_maps, bmax = make_in_maps(x, edge_index, edge_weight, W, b)
    nc = get_program(bmax)
    res = run_bass_kernel_spmd(nc, in_maps, core_ids=list(range(N_CORES)))
    out = np.concatenate([res.results[k]["out"] for k in range(N_CORES)], axis=0)
    out = np.asarray(out, dtype=np.float32)  # [N_NODES, EMBED]
    # unsqueeze(1)/unsqueeze(3) + repeat is a pure broadcast: do it on host
    return np.broadcast_to(out[:, None, :, None],
                           (N_NODES, HEADS, EMBED, REP))
